# revision 33
# baseline (speedup 1.0000x reference)
"""TRN2 Bass kernel for nn_MultiHeadAttention (B=4, S=2048, D=512, H=8).

Computation (per reference):
  v_in = LN(seq_v) ; q = seq_q@W1.T ; k = seq_k@W2.T ; v = v_in@W3.T
  scores[b,h,i,j] = k_i . q_j ; attn = softmax_j(scores) ; out = attn @ v
  out = LN(out + v_in)

Sharding (zero-communication): core c -> (batch b=c//2, i-half=c%2).
Each core computes all 8 heads for its 1024 output rows (the "i" index,
which indexes K rows), needing full q/v (all j) for its batch and the
i-half slice of k. The j axis is permuted host-side (own half first) so
one SPMD program serves all cores; softmax over j is permutation
invariant and the residual rows are j-tiles 0..7 by construction.

v2 design notes:
  - pre-LN of seq_v folded into host prep (ships vinT bf16 + vinres f32)
  - q/k path in fp16 (full-rate PE, ~5e-4 mantissa), v/p path in bf16
    (range needed for unnormalized exp), accumulation always f32 PSUM
  - all projections (q/k/v) are folded into the attention block stream
    as burst slots so TensorE proj work hides under ScalarE's exp pace
  - exp without max-subtraction (f32 exp range suffices; p stored bf16)
  - denominator = ones column appended to v (65-wide PV output)
  - ScalarE runs ONLY exps (plus one dummy exp to preload the table);
    PSUM->SBUF copies are on DVE; final LN rsqrt via DVE-only Newton
"""

import numpy as np
import ml_dtypes

B, S, D, H = 4, 2048, 512, 8
HD = D // H  # 64
EPS = 1e-5
NCORES = 8
IH = S // 2          # 1024 output rows per core
NT = S // 128        # 16 j token-tiles
ITILES = IH // 128   # 8 i-tiles
DT = D // 128        # 4 d-tiles (head pairs)
ET = D // 128        # 4 e-tiles (contraction)

_cache = {}


def _build(has_gamma: bool, has_beta: bool):
    import concourse.bacc as bacc
    import concourse.mybir as mybir
    import concourse.tile as tile
    from concourse.masks import make_identity

    f32 = mybir.dt.float32
    f16 = mybir.dt.float16
    bf16 = mybir.dt.bfloat16
    Alu = mybir.AluOpType
    Act = mybir.ActivationFunctionType

    nc = bacc.Bacc(None, target_bir_lowering=False)

    # every input chunk is its own contiguous dram tensor: contiguous
    # blobs move at ~119GB/s per queue (4KB packets) vs ~45GB/s for
    # strided slices (1KB packets)
    sqTc = [
        nc.dram_tensor(f"sq{j}T", [128, ET, 512], f16, kind="ExternalInput")
        for j in range(4)
    ]
    skTc = [
        nc.dram_tensor(f"sk{i}T", [128, ET, 512], f16, kind="ExternalInput")
        for i in range(2)
    ]
    vinTc = [
        nc.dram_tensor(f"vin{c}T", [128, ET, 512], bf16, kind="ExternalInput")
        for c in range(4)
    ]
    vresc = [
        nc.dram_tensor(f"vres{c}", [128, 4, D], bf16, kind="ExternalInput")
        for c in range(2)
    ]
    w1T = nc.dram_tensor("w1T", [128, ET, D], f16, kind="ExternalInput")
    w2T = nc.dram_tensor("w2T", [128, ET, D], f16, kind="ExternalInput")
    w3T = nc.dram_tensor("w3T", [128, ET, D], bf16, kind="ExternalInput")
    gamma = nc.dram_tensor("gamma", [1, D], f32, kind="ExternalInput")
    beta = nc.dram_tensor("beta", [1, D], f32, kind="ExternalInput")
    outc = [
        nc.dram_tensor(f"out{it}", [128, D], f32, kind="ExternalOutput")
        for it in range(ITILES)
    ]

    def bcast(dram_ap):
        import concourse.bass as bass

        return bass.AP(
            tensor=dram_ap.tensor,
            offset=dram_ap.offset,
            ap=[[0, 128], [1, D]],
        )

    ts = lambda i, sz: slice(i * sz, (i + 1) * sz)

    with tile.TileContext(nc) as tc:
        with (
            tc.tile_pool(name="const", bufs=1) as const,
            tc.tile_pool(name="persist", bufs=1) as persist,
        ):
            # input streams spread across 3 engine DMA queues (each queue
            # transfers serially at ~100GB/s; parallelism is across queues).
            # Separate tiles per chunk (dep tracking is tile-granular).
            wq_pool = tc.alloc_tile_pool(name="wq", bufs=1)
            w1_sb = wq_pool.tile([128, ET, D], f16, tag="w1")
            w2_sb = wq_pool.tile([128, ET, D], f16, tag="w2")
            w3_sb = wq_pool.tile([128, ET, D], bf16, tag="w3")
            sqc = [
                persist.tile([128, ET, 512], f16, tag=f"sq{jc}", name=f"sqc{jc}")
                for jc in range(4)
            ]
            skc = [
                persist.tile([128, ET, 512], f16, tag=f"sk{ic}", name=f"skc{ic}")
                for ic in range(2)
            ]
            vinc = [
                persist.tile([128, ET, 512], bf16, tag=f"vin{c}", name=f"vinc{c}")
                for c in range(4)
            ]
            vinres = [
                persist.tile([128, 4, D], bf16, tag=f"vres{c}", name=f"vinres{c}")
                for c in range(2)
            ]
            # Each queue streams serially at ~68GB/s (4KB packets); the
            # three queues run in parallel. Order each queue by deadline;
            # late chunks are split into partition-halves across two queues
            # so every 3.7us slot carries the most urgent bytes.
            ident = const.tile([128, 128], f32, tag="ident")
            make_identity(nc, ident)
            H64 = slice(0, 64)
            H128 = slice(64, 128)
            nc.sync.dma_start(w1_sb, w1T[:])
            nc.sync.dma_start(w2_sb[H64], w2T[H64])
            nc.sync.dma_start(sqc[1][H64], sqTc[1][H64])
            nc.sync.dma_start(sqc[2][H64], sqTc[2][H64])
            nc.sync.dma_start(sqc[3][H64], sqTc[3][H64])
            nc.sync.dma_start(vinc[2][H64], vinTc[2][H64])
            nc.sync.dma_start(skc[1][H64], skTc[1][H64])
            nc.sync.dma_start(vinres[0], vresc[0][:])
            nc.scalar.dma_start(sqc[0], sqTc[0][:])
            nc.scalar.dma_start(w2_sb[H128], w2T[H128])
            dxi = const.tile([128, 1], f32, tag="dxi")
            nc.vector.memset(dxi, 0.0)
            dxo = const.tile([128, 1], f32, tag="dxo")
            nc.scalar.activation(dxo, dxi, Act.Exp)
            nc.scalar.dma_start(sqc[1][H128], sqTc[1][H128])
            nc.scalar.dma_start(sqc[2][H128], sqTc[2][H128])
            nc.scalar.dma_start(sqc[3][H128], sqTc[3][H128])
            nc.scalar.dma_start(vinc[2][H128], vinTc[2][H128])
            nc.scalar.dma_start(skc[1][H128], skTc[1][H128])
            nc.scalar.dma_start(vinres[1], vresc[1][:])
            nc.gpsimd.dma_start(skc[0], skTc[0][:])
            nc.gpsimd.dma_start(vinc[0], vinTc[0][:])
            nc.gpsimd.dma_start(w3_sb, w3T[:])
            nc.gpsimd.dma_start(vinc[1], vinTc[1][:])
            nc.gpsimd.dma_start(vinc[3], vinTc[3][:])
            if has_gamma:
                gammab = const.tile([128, D], f32, tag="gammab")
                nc.gpsimd.dma_start(gammab, bcast(gamma[:]))
            if has_beta:
                betab = const.tile([128, D], f32, tag="betab")
                nc.gpsimd.dma_start(betab, bcast(beta[:]))

            # persistent intermediates
            qT_sb = persist.tile([128, DT, S], f16, tag="qT")
            kT_sb = persist.tile([128, DT, IH], f16, tag="kT")
            vaug = persist.tile([128, NT, H, 65], bf16, tag="vaug")
            outT_e = persist.tile([65, DT, IH], f32, tag="outTe")
            outT_o = persist.tile([65, DT, IH], f32, tag="outTo")
            y_c = [
                persist.tile([128, 4, D], f32, tag=f"y{c}", name=f"y{c}") for c in range(2)
            ]

            onesc = const.tile([128, NT * H], f32, tag="onesc")
            nc.vector.memset(onesc, 1.0)
            nc.vector.tensor_copy(
                vaug[:, :, :, 64],
                onesc.rearrange("p (a b) -> p a b", a=NT),
            )

            # PSUM pools: sps 4 banks + ops 2 + jpp 1 + vpp 1 = 8.
            # jpp doubles as the finalize-transpose scratch bank.
            sps = tc.alloc_tile_pool(name="sps", bufs=2, space="PSUM")
            ops = tc.alloc_tile_pool(name="ops", bufs=1, space="PSUM")
            jpp = tc.alloc_tile_pool(name="jpp", bufs=1, space="PSUM")
            vpp = tc.alloc_tile_pool(name="vpp", bufs=1, space="PSUM")
            ppool = tc.alloc_tile_pool(name="ppool", bufs=13)
            fin = tc.alloc_tile_pool(name="fin", bufs=4)
            fsc = tc.alloc_tile_pool(name="fsc", bufs=8)

            def qproj(t, jc):
                ps = jpp.tile([128, 512], f32, tag="jp")
                for e in range(ET):
                    nc.tensor.matmul(
                        ps,
                        w1_sb[:, e, ts(t, 128)],
                        sqc[jc][:, e, :],
                        start=(e == 0),
                        stop=(e == ET - 1),
                    )
                nc.vector.tensor_copy(qT_sb[:, t, ts(jc, 512)], ps)

            def kproj(t, ic):
                ps = jpp.tile([128, 512], f32, tag="jp")
                for e in range(ET):
                    nc.tensor.matmul(
                        ps,
                        w2_sb[:, e, ts(t, 128)],
                        skc[ic][:, e, :],
                        start=(e == 0),
                        stop=(e == ET - 1),
                    )
                nc.vector.tensor_copy(kT_sb[:, t, ts(ic, 512)], ps)

            def vproj_half(jt, h):
                psf = vpp.tile([128, 512], f32, tag="vp")
                ps = psf[:, 0:256]
                for e in range(ET):
                    nc.tensor.matmul(
                        ps,
                        vinc[jt // 4][:, e, ts(jt % 4, 128)],
                        w3_sb[:, e, ts(h, 256)],
                        start=(e == 0),
                        stop=(e == ET - 1),
                    )
                nc.vector.tensor_copy(
                    vaug[:, jt, 4 * h : 4 * h + 4, 0:64],
                    ps.rearrange("p (h d) -> p h d", h=4),
                )

            def vproj_full(jt):
                ps = vpp.tile([128, 512], f32, tag="vp")
                for e in range(ET):
                    nc.tensor.matmul(
                        ps,
                        vinc[jt // 4][:, e, ts(jt % 4, 128)],
                        w3_sb[:, e, :],
                        start=(e == 0),
                        stop=(e == ET - 1),
                    )
                nc.vector.tensor_copy(
                    vaug[:, jt, :, 0:64],
                    ps.rearrange("p (h d) -> p h d", h=8),
                )

            def attn_block(t, ib, extras=None):
                o_e = ops.tile([65, 512], f32, tag="oe")
                o_o = ops.tile([65, 512], f32, tag="oo")

                def pv(jt, p):
                    nc.tensor.matmul(
                        o_e,
                        vaug[:, jt, 2 * t, :],
                        p[:, 0:512],
                        start=(jt == 0),
                        stop=(jt == NT - 1),
                    )
                    nc.tensor.matmul(
                        o_o,
                        vaug[:, jt, 2 * t + 1, :],
                        p[:, 512:1024],
                        start=(jt == 0),
                        stop=(jt == NT - 1),
                    )

                prev = None
                for jt in range(NT):
                    s = sps.tile([128, 1024], f32, tag="s")
                    nc.tensor.matmul(
                        s[:, 0:512],
                        qT_sb[0:64, t, ts(jt, 128)],
                        kT_sb[0:64, t, ts(ib, 512)],
                        start=True,
                        stop=True,
                    )
                    nc.tensor.matmul(
                        s[:, 512:1024],
                        qT_sb[64:128, t, ts(jt, 128)],
                        kT_sb[64:128, t, ts(ib, 512)],
                        start=True,
                        stop=True,
                    )
                    p = ppool.tile([128, 1024], bf16, tag="p")
                    nc.scalar.activation(p, s, Act.Exp)
                    if extras is not None:
                        for th in extras.get(jt, ()):
                            th()
                    if prev is not None:
                        pv(*prev)
                    prev = (jt, p)
                pv(*prev)
                nc.vector.tensor_copy(outT_e[:, t, ts(ib, 512)], o_e)
                nc.vector.tensor_copy(outT_o[:, t, ts(ib, 512)], o_o)

            def fin_part1(it, t, fps):
                # assemble divided attention output chunks into y
                for src, off in ((outT_e, 0), (outT_o, 64)):
                    tp = fps.tile([128, 65], f32, tag="tp")
                    nc.tensor.transpose(
                        tp,
                        src[0:65, t, ts(it, 128)],
                        ident[0:65, 0:65],
                    )
                    rc = fsc.tile([128, 1], f32, tag="rc")
                    nc.vector.reciprocal(rc, tp[:, 64:65])
                    col = t * 128 + off
                    nc.vector.tensor_scalar_mul(
                        y_c[it // 4][:, it % 4, col : col + 64],
                        tp[:, 0:64],
                        rc,
                    )

            def fin_part2(its, tail=False):
                i32 = mybir.dt.int32
                mvs = []
                for it in its:
                    y = y_c[it // 4][:, it % 4, :]
                    nc.vector.tensor_add(y, y, vinres[it // 4][:, it % 4, :])
                    st = fin.tile([128, 6], f32, tag="st")
                    nc.vector.bn_stats(st, y)
                    mv = fin.tile([128, 2], f32, tag="mv")
                    nc.vector.bn_aggr(mv, st)
                    mvs.append(mv)
                # batched rstd via DVE-only Newton iteration (ScalarE is
                # reserved for Exp; avoids an act-table switch)
                n = len(its)
                ve = fin.tile([128, n], f32, tag="ve")
                for i, mv in enumerate(mvs):
                    nc.vector.tensor_scalar_add(ve[:, i : i + 1], mv[:, 1:2], EPS)
                rstd2 = fin.tile([128, n], f32, tag="rstd2")
                nc.vector.tensor_scalar(
                    out=rstd2.bitcast(i32),
                    in0=ve.bitcast(i32),
                    scalar1=1,
                    scalar2=None,
                    op0=Alu.logical_shift_right,
                )
                nc.vector.tensor_scalar(
                    out=rstd2.bitcast(i32),
                    in0=rstd2.bitcast(i32),
                    scalar1=-1,
                    scalar2=0x5F3759DF,
                    op0=Alu.mult,
                    op1=Alu.add,
                )
                tmp1 = fin.tile([128, n], f32, tag="tmp1")
                for _ in range(2):
                    nc.vector.tensor_mul(tmp1, rstd2, rstd2)
                    nc.vector.tensor_mul(tmp1, tmp1, ve)
                    nc.vector.tensor_scalar(
                        out=tmp1,
                        in0=tmp1,
                        scalar1=-0.5,
                        scalar2=1.5,
                        op0=Alu.mult,
                        op1=Alu.add,
                    )
                    nc.vector.tensor_mul(rstd2, rstd2, tmp1)
                for i, it in enumerate(its):
                    y = y_c[it // 4][:, it % 4, :]
                    if tail and not has_gamma and not has_beta:
                        # ScalarE is idle after the last exp: do the final
                        # affine there, in parallel with DVE's stats chain
                        nb = fin.tile([128, 1], f32, tag="nb")
                        nc.vector.tensor_scalar(
                            out=nb,
                            in0=mvs[i][:, 0:1],
                            scalar1=rstd2[:, i : i + 1],
                            scalar2=-1.0,
                            op0=Alu.mult,
                            op1=Alu.mult,
                        )
                        yo = fin.tile([128, D], f32, tag="yo")
                        nc.scalar.activation(
                            yo,
                            y,
                            Act.Identity,
                            bias=nb,
                            scale=rstd2[:, i : i + 1],
                        )
                        y = yo
                    else:
                        nc.vector.tensor_scalar(
                            out=y,
                            in0=y,
                            scalar1=mvs[i][:, 0:1],
                            scalar2=rstd2[:, i : i + 1],
                            op0=Alu.subtract,
                            op1=Alu.mult,
                        )
                        if has_gamma:
                            nc.vector.tensor_mul(y, y, gammab)
                        if has_beta:
                            nc.gpsimd.tensor_add(y, y, betab)
                    dq = nc.sync if it % 2 == 0 else nc.gpsimd
                    dq.dma_start(outc[it][:], y)

            # ---- PE warmup: the tensor engine p-state ramps to full
            # clock only after ~3us of continuous execution; burn dummy
            # ident matmuls (no DMA deps) while the first inputs stream.
            # Alternate the two priming PSUM pools so every real priming
            # matmul chains behind a dummy (pool-slot WAW) - otherwise the
            # scheduler hoists a DMA-gated real matmul to the queue head
            # and it blocks the warmup entirely. ----
            for wi in range(10):
                wps = (jpp if wi % 2 == 0 else vpp).tile(
                    [128, 512], f32, tag="jp" if wi % 2 == 0 else "vp"
                )
                nc.tensor.matmul(
                    wps[:, 0:128], ident, ident, start=True, stop=True
                )

            # ---- priming: first q/k tiles only; v tiles are deferred
            # into block 0 (their inputs arrive last) ----
            qproj(0, 0)
            kproj(0, 0)

            # ---- block 0: scores/exp start as soon as q/k are up; the
            # v-projection and PV consumption trail by VD/PD iterations to
            # ride out the input DMA stream, catching up in an epilogue ----
            VD, PD = 9, 11

            def attn_block0():
                t = 0
                o_e = ops.tile([65, 512], f32, tag="oe")
                o_o = ops.tile([65, 512], f32, tag="oo")

                def pv(jt, p):
                    nc.tensor.matmul(
                        o_e,
                        vaug[:, jt, 0, :],
                        p[:, 0:512],
                        start=(jt == 0),
                        stop=(jt == NT - 1),
                    )
                    nc.tensor.matmul(
                        o_o,
                        vaug[:, jt, 1, :],
                        p[:, 512:1024],
                        start=(jt == 0),
                        stop=(jt == NT - 1),
                    )

                bursts = {
                    3: lambda: qproj(0, 1),
                    6: lambda: qproj(0, 2),
                    8: lambda: qproj(0, 3),
                    12: lambda: qproj(1, 0),
                    14: lambda: kproj(1, 0),
                }
                pend = []
                for jt in range(NT):
                    s = sps.tile([128, 1024], f32, tag="s")
                    nc.tensor.matmul(
                        s[:, 0:512],
                        qT_sb[0:64, t, ts(jt, 128)],
                        kT_sb[0:64, t, ts(0, 512)],
                        start=True,
                        stop=True,
                    )
                    nc.tensor.matmul(
                        s[:, 512:1024],
                        qT_sb[64:128, t, ts(jt, 128)],
                        kT_sb[64:128, t, ts(0, 512)],
                        start=True,
                        stop=True,
                    )
                    p = ppool.tile([128, 1024], bf16, tag="p")
                    nc.scalar.activation(p, s, Act.Exp)
                    pend.append((jt, p))
                    if jt >= VD:
                        vproj_full(jt - VD)
                    if jt in bursts:
                        bursts[jt]()
                    if jt >= PD:
                        pv(*pend.pop(0))
                # epilogue: finish the trailing v-projections and PVs,
                # alternating the two free PSUM banks to avoid ring stalls
                vq = list(range(NT - VD, NT))
                k = 0
                for n_, v_jt in enumerate(vq):
                    ps = (vpp if n_ % 2 == 0 else jpp).tile(
                        [128, 512], f32, tag="vp" if n_ % 2 == 0 else "jp"
                    )
                    for e in range(ET):
                        nc.tensor.matmul(
                            ps,
                            vinc[v_jt // 4][:, e, ts(v_jt % 4, 128)],
                            w3_sb[:, e, :],
                            start=(e == 0),
                            stop=(e == ET - 1),
                        )
                    nc.vector.tensor_copy(
                        vaug[:, v_jt, :, 0:64],
                        ps.rearrange("p (h d) -> p h d", h=8),
                    )
                    if k < len(pend) and pend[k][0] <= v_jt - 2:
                        pv(*pend[k])
                        k += 1
                while k < len(pend):
                    pv(*pend[k])
                    k += 1
                nc.vector.tensor_copy(outT_e[:, 0, ts(0, 512)], o_e)
                nc.vector.tensor_copy(outT_o[:, 0, ts(0, 512)], o_o)

            ex1 = {
                3: [lambda: qproj(1, 1)],
                5: [lambda: qproj(1, 2)],
                7: [lambda: qproj(1, 3)],
                9: [lambda: qproj(2, 0)],
                11: [lambda: kproj(2, 0)],
            }
            ex2 = {
                1: [lambda: kproj(0, 1)],
                3: [lambda: qproj(2, 1)],
                5: [lambda: qproj(2, 2)],
                7: [lambda: qproj(2, 3)],
                9: [lambda: qproj(3, 0)],
                11: [lambda: kproj(3, 0)],
            }
            ex3 = {
                1: [lambda: kproj(1, 1)],
                3: [lambda: qproj(3, 1)],
                5: [lambda: qproj(3, 2)],
                7: [lambda: qproj(3, 3)],
                9: [lambda: kproj(2, 1)],
                11: [lambda: kproj(3, 1)],
            }

            attn_block0()
            attn_block(1, 0, ex1)
            vpp.release()
            attn_block(2, 0, ex2)
            attn_block(3, 0, ex3)
            jpp.release()
            fps = tc.alloc_tile_pool(name="fps", bufs=2, space="PSUM")

            # finalize ib=0 rows while ib=1 attention runs (part2 issued
            # after the first ib=1 block so its DVE work lands in a window
            # where DVE is otherwise idle)
            for it in range(4):
                for t in range(DT):
                    fin_part1(it, t, fps)

            for t in range(DT):
                attn_block(t, 1)
                if t == 0:
                    fin_part2([0, 1, 2, 3])
                for it in range(4, ITILES):
                    fin_part1(it, t, fps)
            fin_part2([4, 5, 6, 7], tail=True)

            fps.release()
            fsc.release()
            fin.release()
            ppool.release()
            ops.release()
            sps.release()
            wq_pool.release()

    nc.compile()
    return nc


def _to_tiles_T(x, dtype):
    # [N, 512] -> [128, 4, N] : out[p, t, n] = x[n, 128*t + p]
    n = x.shape[0]
    return np.ascontiguousarray(
        x.T.reshape(ET, 128, n).transpose(1, 0, 2).astype(dtype)
    )


def _w_tiles(w, dtype):
    # [512, 512] (e, d) -> [128, 4, 512] : out[p, t, d] = w[128*t + p, d]
    return np.ascontiguousarray(
        w.reshape(ET, 128, D).transpose(1, 0, 2).astype(dtype)
    )


def kernel(seq_k, seq_q, seq_v, W1, W2, W3, gamma, beta, _trace=False):
    bf16 = ml_dtypes.bfloat16
    seq_k = np.asarray(seq_k, dtype=np.float32)
    seq_q = np.asarray(seq_q, dtype=np.float32)
    seq_v = np.asarray(seq_v, dtype=np.float32)
    W1 = np.asarray(W1, dtype=np.float32)
    W2 = np.asarray(W2, dtype=np.float32)
    W3 = np.asarray(W3, dtype=np.float32)
    gamma = np.asarray(gamma, dtype=np.float32)
    beta = np.asarray(beta, dtype=np.float32)

    has_gamma = bool(np.any(gamma != 1.0))
    has_beta = bool(np.any(beta != 0.0))

    key = (has_gamma, has_beta)
    if key not in _cache:
        _cache[key] = _build(has_gamma, has_beta)
    nc = _cache[key]

    from concourse import bass_utils

    # host prep: pre-LN of v (the module's is_layer_norm input transform)
    mu = seq_v.mean(-1, keepdims=True)
    var = ((seq_v - mu) ** 2).mean(-1, keepdims=True)
    v_in = (seq_v - mu) / np.sqrt(var + EPS) * gamma + beta

    w1t = _w_tiles(np.ascontiguousarray(W1.T), np.float16)
    w2t = _w_tiles(np.ascontiguousarray(W2.T), np.float16)
    w3t = _w_tiles(np.ascontiguousarray(W3.T), bf16)
    gam = np.ascontiguousarray(gamma[None, :], dtype=np.float32)
    bet = np.ascontiguousarray(beta[None, :], dtype=np.float32)

    def chunks(x, n):
        # [128, 4, n*512] -> n contiguous [128, 4, 512] arrays
        return [
            np.ascontiguousarray(x[:, :, 512 * i : 512 * (i + 1)])
            for i in range(n)
        ]

    in_maps = []
    for c in range(NCORES):
        b, half = divmod(c, 2)
        lo, hi = half * IH, half * IH + IH
        perm = np.r_[lo:hi, 0:lo, hi:S]
        vresf = v_in[b, lo:hi].reshape(2, 4, 128, D).transpose(0, 2, 1, 3)
        m = {
            "w1T": w1t,
            "w2T": w2t,
            "w3T": w3t,
            "gamma": gam,
            "beta": bet,
        }
        for j, a in enumerate(chunks(_to_tiles_T(seq_q[b][perm], np.float16), 4)):
            m[f"sq{j}T"] = a
        for i, a in enumerate(chunks(_to_tiles_T(seq_k[b, lo:hi], np.float16), 2)):
            m[f"sk{i}T"] = a
        for j, a in enumerate(chunks(_to_tiles_T(v_in[b][perm], bf16), 4)):
            m[f"vin{j}T"] = a
        for j in range(2):
            m[f"vres{j}"] = np.ascontiguousarray(vresf[j]).astype(bf16)
        in_maps.append(m)

    res = bass_utils.run_bass_kernel_spmd(
        nc, in_maps, core_ids=list(range(NCORES)), trace=_trace
    )
    global _last_run
    _last_run = res

    full = np.empty((B, S, D), dtype=np.float32)
    for c in range(NCORES):
        b, half = divmod(c, 2)
        for it in range(ITILES):
            o = res.results[c][f"out{it}"]  # [128, 512]
            full[b, half * IH + 128 * it : half * IH + 128 * (it + 1)] = o
    return full


_last_run = None


# revision 34
# speedup vs baseline: 1.0224x; 1.0224x over previous
"""TRN2 Bass kernel for nn_MultiHeadAttention (B=4, S=2048, D=512, H=8).

Computation (per reference):
  v_in = LN(seq_v) ; q = seq_q@W1.T ; k = seq_k@W2.T ; v = v_in@W3.T
  scores[b,h,i,j] = k_i . q_j ; attn = softmax_j(scores) ; out = attn @ v
  out = LN(out + v_in)

Sharding (zero-communication): core c -> (batch b=c//2, i-half=c%2).
Each core computes all 8 heads for its 1024 output rows (the "i" index,
which indexes K rows), needing full q/v (all j) for its batch and the
i-half slice of k. The j axis is permuted host-side (own half first) so
one SPMD program serves all cores; softmax over j is permutation
invariant and the residual rows are j-tiles 0..7 by construction.

v2 design notes:
  - pre-LN of seq_v folded into host prep (ships vinT bf16 + vinres f32)
  - q/k path in fp16 (full-rate PE, ~5e-4 mantissa), v/p path in bf16
    (range needed for unnormalized exp), accumulation always f32 PSUM
  - all projections (q/k/v) are folded into the attention block stream
    as burst slots so TensorE proj work hides under ScalarE's exp pace
  - exp without max-subtraction (f32 exp range suffices; p stored bf16)
  - denominator = ones column appended to v (65-wide PV output)
  - ScalarE runs ONLY exps (plus one dummy exp to preload the table);
    PSUM->SBUF copies are on DVE; final LN rsqrt via DVE-only Newton
"""

import numpy as np
import ml_dtypes

B, S, D, H = 4, 2048, 512, 8
HD = D // H  # 64
EPS = 1e-5
NCORES = 8
IH = S // 2          # 1024 output rows per core
NT = S // 128        # 16 j token-tiles
ITILES = IH // 128   # 8 i-tiles
DT = D // 128        # 4 d-tiles (head pairs)
ET = D // 128        # 4 e-tiles (contraction)

_cache = {}


def _build(has_gamma: bool, has_beta: bool):
    import concourse.bacc as bacc
    import concourse.mybir as mybir
    import concourse.tile as tile
    from concourse.masks import make_identity

    f32 = mybir.dt.float32
    f16 = mybir.dt.float16
    bf16 = mybir.dt.bfloat16
    Alu = mybir.AluOpType
    Act = mybir.ActivationFunctionType

    nc = bacc.Bacc(None, target_bir_lowering=False)

    # every input chunk is its own contiguous dram tensor: contiguous
    # blobs move at ~119GB/s per queue (4KB packets) vs ~45GB/s for
    # strided slices (1KB packets)
    sqTc = [
        nc.dram_tensor(f"sq{j}T", [128, ET, 512], f16, kind="ExternalInput")
        for j in range(4)
    ]
    skTc = [
        nc.dram_tensor(f"sk{i}T", [128, ET, 512], f16, kind="ExternalInput")
        for i in range(2)
    ]
    vinTc = [
        nc.dram_tensor(f"vin{c}T", [128, ET, 512], bf16, kind="ExternalInput")
        for c in range(4)
    ]
    vresc = [
        nc.dram_tensor(f"vres{c}", [128, 4, D], bf16, kind="ExternalInput")
        for c in range(2)
    ]
    w1T = nc.dram_tensor("w1T", [128, ET, D], f16, kind="ExternalInput")
    w2T = nc.dram_tensor("w2T", [128, ET, D], f16, kind="ExternalInput")
    w3T = nc.dram_tensor("w3T", [128, ET, D], bf16, kind="ExternalInput")
    gamma = nc.dram_tensor("gamma", [1, D], f32, kind="ExternalInput")
    beta = nc.dram_tensor("beta", [1, D], f32, kind="ExternalInput")
    outc = [
        nc.dram_tensor(f"out{it}", [128, D], f32, kind="ExternalOutput")
        for it in range(ITILES)
    ]

    def bcast(dram_ap):
        import concourse.bass as bass

        return bass.AP(
            tensor=dram_ap.tensor,
            offset=dram_ap.offset,
            ap=[[0, 128], [1, D]],
        )

    ts = lambda i, sz: slice(i * sz, (i + 1) * sz)

    with tile.TileContext(nc) as tc:
        with (
            tc.tile_pool(name="const", bufs=1) as const,
            tc.tile_pool(name="persist", bufs=1) as persist,
        ):
            # input streams spread across 3 engine DMA queues (each queue
            # transfers serially at ~100GB/s; parallelism is across queues).
            # Separate tiles per chunk (dep tracking is tile-granular).
            wq_pool = tc.alloc_tile_pool(name="wq", bufs=1)
            w1_sb = wq_pool.tile([128, ET, D], f16, tag="w1")
            w2_sb = wq_pool.tile([128, ET, D], f16, tag="w2")
            w3_sb = wq_pool.tile([128, ET, D], bf16, tag="w3")
            sqc = [
                persist.tile([128, ET, 512], f16, tag=f"sq{jc}", name=f"sqc{jc}")
                for jc in range(4)
            ]
            skc = [
                persist.tile([128, ET, 512], f16, tag=f"sk{ic}", name=f"skc{ic}")
                for ic in range(2)
            ]
            vinc = [
                persist.tile([128, ET, 512], bf16, tag=f"vin{c}", name=f"vinc{c}")
                for c in range(4)
            ]
            vinres = [
                persist.tile([128, 4, D], bf16, tag=f"vres{c}", name=f"vinres{c}")
                for c in range(2)
            ]
            # Each queue streams serially at ~68GB/s (4KB packets); the
            # three queues run in parallel. Order each queue by deadline;
            # late chunks are split into partition-halves across two queues
            # so every 3.7us slot carries the most urgent bytes.
            ident = const.tile([128, 128], f32, tag="ident")
            make_identity(nc, ident)
            H64 = slice(0, 64)
            H128 = slice(64, 128)
            nc.sync.dma_start(w1_sb, w1T[:])
            nc.sync.dma_start(vinc[0][H64], vinTc[0][H64])
            nc.sync.dma_start(w2_sb[H64], w2T[H64])
            nc.sync.dma_start(sqc[1][H64], sqTc[1][H64])
            nc.sync.dma_start(sqc[2][H64], sqTc[2][H64])
            nc.sync.dma_start(sqc[3][H64], sqTc[3][H64])
            nc.sync.dma_start(vinc[2][H64], vinTc[2][H64])
            nc.sync.dma_start(skc[1][H64], skTc[1][H64])
            nc.sync.dma_start(vinres[0], vresc[0][:])
            nc.scalar.dma_start(sqc[0], sqTc[0][:])
            nc.scalar.dma_start(vinc[0][H128], vinTc[0][H128])
            dxi = const.tile([128, 1], f32, tag="dxi")
            nc.vector.memset(dxi, 0.0)
            dxo = const.tile([128, 1], f32, tag="dxo")
            nc.scalar.activation(dxo, dxi, Act.Exp)
            nc.scalar.dma_start(w2_sb[H128], w2T[H128])
            nc.scalar.dma_start(sqc[1][H128], sqTc[1][H128])
            nc.scalar.dma_start(sqc[2][H128], sqTc[2][H128])
            nc.scalar.dma_start(sqc[3][H128], sqTc[3][H128])
            nc.scalar.dma_start(vinc[2][H128], vinTc[2][H128])
            nc.scalar.dma_start(skc[1][H128], skTc[1][H128])
            nc.scalar.dma_start(vinres[1], vresc[1][:])
            nc.gpsimd.dma_start(w3_sb, w3T[:])
            nc.gpsimd.dma_start(skc[0], skTc[0][:])
            nc.gpsimd.dma_start(vinc[1], vinTc[1][:])
            nc.gpsimd.dma_start(vinc[3], vinTc[3][:])
            if has_gamma:
                gammab = const.tile([128, D], f32, tag="gammab")
                nc.gpsimd.dma_start(gammab, bcast(gamma[:]))
            if has_beta:
                betab = const.tile([128, D], f32, tag="betab")
                nc.gpsimd.dma_start(betab, bcast(beta[:]))

            # persistent intermediates
            qT_sb = persist.tile([128, DT, S], f16, tag="qT")
            kT_sb = persist.tile([128, DT, IH], f16, tag="kT")
            vaug = persist.tile([128, NT, H, 65], bf16, tag="vaug")
            outT_e = persist.tile([65, DT, IH], f32, tag="outTe")
            outT_o = persist.tile([65, DT, IH], f32, tag="outTo")
            y_c = [
                persist.tile([128, 4, D], f32, tag=f"y{c}", name=f"y{c}") for c in range(2)
            ]

            onesc = const.tile([128, NT * H], f32, tag="onesc")
            nc.vector.memset(onesc, 1.0)
            nc.vector.tensor_copy(
                vaug[:, :, :, 64],
                onesc.rearrange("p (a b) -> p a b", a=NT),
            )

            # PSUM pools: sps 4 banks + ops 2 + jpp 1 + vpp 1 = 8.
            # jpp doubles as the finalize-transpose scratch bank.
            sps = tc.alloc_tile_pool(name="sps", bufs=2, space="PSUM")
            ops = tc.alloc_tile_pool(name="ops", bufs=1, space="PSUM")
            jpp = tc.alloc_tile_pool(name="jpp", bufs=1, space="PSUM")
            vpp = tc.alloc_tile_pool(name="vpp", bufs=1, space="PSUM")
            ppool = tc.alloc_tile_pool(name="ppool", bufs=13)
            fin = tc.alloc_tile_pool(name="fin", bufs=4)
            fsc = tc.alloc_tile_pool(name="fsc", bufs=8)

            def qproj(t, jc):
                ps = jpp.tile([128, 512], f32, tag="jp")
                for e in range(ET):
                    nc.tensor.matmul(
                        ps,
                        w1_sb[:, e, ts(t, 128)],
                        sqc[jc][:, e, :],
                        start=(e == 0),
                        stop=(e == ET - 1),
                    )
                nc.vector.tensor_copy(qT_sb[:, t, ts(jc, 512)], ps)

            def kproj(t, ic):
                ps = jpp.tile([128, 512], f32, tag="jp")
                for e in range(ET):
                    nc.tensor.matmul(
                        ps,
                        w2_sb[:, e, ts(t, 128)],
                        skc[ic][:, e, :],
                        start=(e == 0),
                        stop=(e == ET - 1),
                    )
                nc.vector.tensor_copy(kT_sb[:, t, ts(ic, 512)], ps)

            def vproj_half(jt, h):
                psf = vpp.tile([128, 512], f32, tag="vp")
                ps = psf[:, 0:256]
                for e in range(ET):
                    nc.tensor.matmul(
                        ps,
                        vinc[jt // 4][:, e, ts(jt % 4, 128)],
                        w3_sb[:, e, ts(h, 256)],
                        start=(e == 0),
                        stop=(e == ET - 1),
                    )
                nc.vector.tensor_copy(
                    vaug[:, jt, 4 * h : 4 * h + 4, 0:64],
                    ps.rearrange("p (h d) -> p h d", h=4),
                )

            def vproj_full(jt):
                ps = vpp.tile([128, 512], f32, tag="vp")
                for e in range(ET):
                    nc.tensor.matmul(
                        ps,
                        vinc[jt // 4][:, e, ts(jt % 4, 128)],
                        w3_sb[:, e, :],
                        start=(e == 0),
                        stop=(e == ET - 1),
                    )
                nc.vector.tensor_copy(
                    vaug[:, jt, :, 0:64],
                    ps.rearrange("p (h d) -> p h d", h=8),
                )

            def attn_block(t, ib, extras=None):
                o_e = ops.tile([65, 512], f32, tag="oe")
                o_o = ops.tile([65, 512], f32, tag="oo")

                def pv(jt, p):
                    nc.tensor.matmul(
                        o_e,
                        vaug[:, jt, 2 * t, :],
                        p[:, 0:512],
                        start=(jt == 0),
                        stop=(jt == NT - 1),
                    )
                    nc.tensor.matmul(
                        o_o,
                        vaug[:, jt, 2 * t + 1, :],
                        p[:, 512:1024],
                        start=(jt == 0),
                        stop=(jt == NT - 1),
                    )

                prev = None
                for jt in range(NT):
                    s = sps.tile([128, 1024], f32, tag="s")
                    nc.tensor.matmul(
                        s[:, 0:512],
                        qT_sb[0:64, t, ts(jt, 128)],
                        kT_sb[0:64, t, ts(ib, 512)],
                        start=True,
                        stop=True,
                    )
                    nc.tensor.matmul(
                        s[:, 512:1024],
                        qT_sb[64:128, t, ts(jt, 128)],
                        kT_sb[64:128, t, ts(ib, 512)],
                        start=True,
                        stop=True,
                    )
                    p = ppool.tile([128, 1024], bf16, tag="p")
                    nc.scalar.activation(p, s, Act.Exp)
                    if extras is not None:
                        for th in extras.get(jt, ()):
                            th()
                    if prev is not None:
                        pv(*prev)
                    prev = (jt, p)
                pv(*prev)
                nc.vector.tensor_copy(outT_e[:, t, ts(ib, 512)], o_e)
                nc.vector.tensor_copy(outT_o[:, t, ts(ib, 512)], o_o)

            def fin_part1(it, t, fps):
                # assemble divided attention output chunks into y
                for src, off in ((outT_e, 0), (outT_o, 64)):
                    tp = fps.tile([128, 65], f32, tag="tp")
                    nc.tensor.transpose(
                        tp,
                        src[0:65, t, ts(it, 128)],
                        ident[0:65, 0:65],
                    )
                    rc = fsc.tile([128, 1], f32, tag="rc")
                    nc.vector.reciprocal(rc, tp[:, 64:65])
                    col = t * 128 + off
                    nc.vector.tensor_scalar_mul(
                        y_c[it // 4][:, it % 4, col : col + 64],
                        tp[:, 0:64],
                        rc,
                    )

            def fin_part2(its, tail=False):
                i32 = mybir.dt.int32
                mvs = []
                for it in its:
                    y = y_c[it // 4][:, it % 4, :]
                    nc.vector.tensor_add(y, y, vinres[it // 4][:, it % 4, :])
                    st = fin.tile([128, 6], f32, tag="st")
                    nc.vector.bn_stats(st, y)
                    mv = fin.tile([128, 2], f32, tag="mv")
                    nc.vector.bn_aggr(mv, st)
                    mvs.append(mv)
                # batched rstd via DVE-only Newton iteration (ScalarE is
                # reserved for Exp; avoids an act-table switch)
                n = len(its)
                ve = fin.tile([128, n], f32, tag="ve")
                for i, mv in enumerate(mvs):
                    nc.vector.tensor_scalar_add(ve[:, i : i + 1], mv[:, 1:2], EPS)
                rstd2 = fin.tile([128, n], f32, tag="rstd2")
                nc.vector.tensor_scalar(
                    out=rstd2.bitcast(i32),
                    in0=ve.bitcast(i32),
                    scalar1=1,
                    scalar2=None,
                    op0=Alu.logical_shift_right,
                )
                nc.vector.tensor_scalar(
                    out=rstd2.bitcast(i32),
                    in0=rstd2.bitcast(i32),
                    scalar1=-1,
                    scalar2=0x5F3759DF,
                    op0=Alu.mult,
                    op1=Alu.add,
                )
                tmp1 = fin.tile([128, n], f32, tag="tmp1")
                for _ in range(2):
                    nc.vector.tensor_mul(tmp1, rstd2, rstd2)
                    nc.vector.tensor_mul(tmp1, tmp1, ve)
                    nc.vector.tensor_scalar(
                        out=tmp1,
                        in0=tmp1,
                        scalar1=-0.5,
                        scalar2=1.5,
                        op0=Alu.mult,
                        op1=Alu.add,
                    )
                    nc.vector.tensor_mul(rstd2, rstd2, tmp1)
                for i, it in enumerate(its):
                    y = y_c[it // 4][:, it % 4, :]
                    if tail and not has_gamma and not has_beta:
                        # ScalarE is idle after the last exp: do the final
                        # affine there, in parallel with DVE's stats chain
                        nb = fin.tile([128, 1], f32, tag="nb")
                        nc.vector.tensor_scalar(
                            out=nb,
                            in0=mvs[i][:, 0:1],
                            scalar1=rstd2[:, i : i + 1],
                            scalar2=-1.0,
                            op0=Alu.mult,
                            op1=Alu.mult,
                        )
                        yo = fin.tile([128, D], f32, tag="yo")
                        nc.scalar.activation(
                            yo,
                            y,
                            Act.Identity,
                            bias=nb,
                            scale=rstd2[:, i : i + 1],
                        )
                        y = yo
                    else:
                        nc.vector.tensor_scalar(
                            out=y,
                            in0=y,
                            scalar1=mvs[i][:, 0:1],
                            scalar2=rstd2[:, i : i + 1],
                            op0=Alu.subtract,
                            op1=Alu.mult,
                        )
                        if has_gamma:
                            nc.vector.tensor_mul(y, y, gammab)
                        if has_beta:
                            nc.gpsimd.tensor_add(y, y, betab)
                    dq = nc.sync if it % 2 == 0 else nc.gpsimd
                    dq.dma_start(outc[it][:], y)

            # ---- PE warmup: the tensor engine p-state ramps to full
            # clock only after ~3us of continuous execution; burn dummy
            # ident matmuls (no DMA deps) while the first inputs stream.
            # Alternate the two priming PSUM pools so every real priming
            # matmul chains behind a dummy (pool-slot WAW) - otherwise the
            # scheduler hoists a DMA-gated real matmul to the queue head
            # and it blocks the warmup entirely. ----
            for wi in range(10):
                wps = (jpp if wi % 2 == 0 else vpp).tile(
                    [128, 512], f32, tag="jp" if wi % 2 == 0 else "vp"
                )
                nc.tensor.matmul(
                    wps[:, 0:128], ident, ident, start=True, stop=True
                )

            # ---- priming: first q/k tiles only; v tiles are deferred
            # into block 0 (their inputs arrive last) ----
            qproj(0, 0)
            kproj(0, 0)

            # ---- block 0: scores/exp start as soon as q/k are up; the
            # v-projection and PV consumption trail by VD/PD iterations to
            # ride out the input DMA stream, catching up in an epilogue ----
            VD, PD = 9, 11

            def attn_block0():
                t = 0
                o_e = ops.tile([65, 512], f32, tag="oe")
                o_o = ops.tile([65, 512], f32, tag="oo")

                def pv(jt, p):
                    nc.tensor.matmul(
                        o_e,
                        vaug[:, jt, 0, :],
                        p[:, 0:512],
                        start=(jt == 0),
                        stop=(jt == NT - 1),
                    )
                    nc.tensor.matmul(
                        o_o,
                        vaug[:, jt, 1, :],
                        p[:, 512:1024],
                        start=(jt == 0),
                        stop=(jt == NT - 1),
                    )

                bursts = {
                    3: lambda: qproj(0, 1),
                    6: lambda: qproj(0, 2),
                    10: lambda: qproj(0, 3),
                    12: lambda: qproj(1, 0),
                    14: lambda: kproj(1, 0),
                }
                pend = []
                for jt in range(NT):
                    s = sps.tile([128, 1024], f32, tag="s")
                    nc.tensor.matmul(
                        s[:, 0:512],
                        qT_sb[0:64, t, ts(jt, 128)],
                        kT_sb[0:64, t, ts(0, 512)],
                        start=True,
                        stop=True,
                    )
                    nc.tensor.matmul(
                        s[:, 512:1024],
                        qT_sb[64:128, t, ts(jt, 128)],
                        kT_sb[64:128, t, ts(0, 512)],
                        start=True,
                        stop=True,
                    )
                    p = ppool.tile([128, 1024], bf16, tag="p")
                    nc.scalar.activation(p, s, Act.Exp)
                    pend.append((jt, p))
                    if jt >= VD:
                        vproj_full(jt - VD)
                    if jt in bursts:
                        bursts[jt]()
                    if jt >= PD:
                        pv(*pend.pop(0))
                # epilogue: finish the trailing v-projections and PVs,
                # alternating the two free PSUM banks to avoid ring stalls
                vq = list(range(NT - VD, NT))
                k = 0
                for n_, v_jt in enumerate(vq):
                    ps = (vpp if n_ % 2 == 0 else jpp).tile(
                        [128, 512], f32, tag="vp" if n_ % 2 == 0 else "jp"
                    )
                    for e in range(ET):
                        nc.tensor.matmul(
                            ps,
                            vinc[v_jt // 4][:, e, ts(v_jt % 4, 128)],
                            w3_sb[:, e, :],
                            start=(e == 0),
                            stop=(e == ET - 1),
                        )
                    nc.vector.tensor_copy(
                        vaug[:, v_jt, :, 0:64],
                        ps.rearrange("p (h d) -> p h d", h=8),
                    )
                    if k < len(pend) and pend[k][0] <= v_jt - 2:
                        pv(*pend[k])
                        k += 1
                while k < len(pend):
                    pv(*pend[k])
                    k += 1
                nc.vector.tensor_copy(outT_e[:, 0, ts(0, 512)], o_e)
                nc.vector.tensor_copy(outT_o[:, 0, ts(0, 512)], o_o)

            ex1 = {
                3: [lambda: qproj(1, 1)],
                5: [lambda: qproj(1, 2)],
                7: [lambda: qproj(1, 3)],
                9: [lambda: qproj(2, 0)],
                11: [lambda: kproj(2, 0)],
            }
            ex2 = {
                1: [lambda: kproj(0, 1)],
                3: [lambda: qproj(2, 1)],
                5: [lambda: qproj(2, 2)],
                7: [lambda: qproj(2, 3)],
                9: [lambda: qproj(3, 0)],
                11: [lambda: kproj(3, 0)],
            }
            ex3 = {
                1: [lambda: kproj(1, 1)],
                3: [lambda: qproj(3, 1)],
                5: [lambda: qproj(3, 2)],
                7: [lambda: qproj(3, 3)],
                9: [lambda: kproj(2, 1)],
                11: [lambda: kproj(3, 1)],
            }

            attn_block0()
            attn_block(1, 0, ex1)
            vpp.release()
            attn_block(2, 0, ex2)
            attn_block(3, 0, ex3)
            jpp.release()
            fps = tc.alloc_tile_pool(name="fps", bufs=2, space="PSUM")

            # finalize ib=0 rows while ib=1 attention runs (part2 issued
            # after the first ib=1 block so its DVE work lands in a window
            # where DVE is otherwise idle)
            for it in range(4):
                for t in range(DT):
                    fin_part1(it, t, fps)

            for t in range(DT):
                attn_block(t, 1)
                if t == 0:
                    fin_part2([0, 1, 2, 3])
                for it in range(4, ITILES):
                    fin_part1(it, t, fps)
            fin_part2([4, 5, 6, 7], tail=True)

            fps.release()
            fsc.release()
            fin.release()
            ppool.release()
            ops.release()
            sps.release()
            wq_pool.release()

    nc.compile()
    return nc


def _to_tiles_T(x, dtype):
    # [N, 512] -> [128, 4, N] : out[p, t, n] = x[n, 128*t + p]
    n = x.shape[0]
    return np.ascontiguousarray(
        x.T.reshape(ET, 128, n).transpose(1, 0, 2).astype(dtype)
    )


def _w_tiles(w, dtype):
    # [512, 512] (e, d) -> [128, 4, 512] : out[p, t, d] = w[128*t + p, d]
    return np.ascontiguousarray(
        w.reshape(ET, 128, D).transpose(1, 0, 2).astype(dtype)
    )


def kernel(seq_k, seq_q, seq_v, W1, W2, W3, gamma, beta, _trace=False):
    bf16 = ml_dtypes.bfloat16
    seq_k = np.asarray(seq_k, dtype=np.float32)
    seq_q = np.asarray(seq_q, dtype=np.float32)
    seq_v = np.asarray(seq_v, dtype=np.float32)
    W1 = np.asarray(W1, dtype=np.float32)
    W2 = np.asarray(W2, dtype=np.float32)
    W3 = np.asarray(W3, dtype=np.float32)
    gamma = np.asarray(gamma, dtype=np.float32)
    beta = np.asarray(beta, dtype=np.float32)

    has_gamma = bool(np.any(gamma != 1.0))
    has_beta = bool(np.any(beta != 0.0))

    key = (has_gamma, has_beta)
    if key not in _cache:
        _cache[key] = _build(has_gamma, has_beta)
    nc = _cache[key]

    from concourse import bass_utils

    # host prep: pre-LN of v (the module's is_layer_norm input transform)
    mu = seq_v.mean(-1, keepdims=True)
    var = ((seq_v - mu) ** 2).mean(-1, keepdims=True)
    v_in = (seq_v - mu) / np.sqrt(var + EPS) * gamma + beta

    w1t = _w_tiles(np.ascontiguousarray(W1.T), np.float16)
    w2t = _w_tiles(np.ascontiguousarray(W2.T), np.float16)
    w3t = _w_tiles(np.ascontiguousarray(W3.T), bf16)
    gam = np.ascontiguousarray(gamma[None, :], dtype=np.float32)
    bet = np.ascontiguousarray(beta[None, :], dtype=np.float32)

    def chunks(x, n):
        # [128, 4, n*512] -> n contiguous [128, 4, 512] arrays
        return [
            np.ascontiguousarray(x[:, :, 512 * i : 512 * (i + 1)])
            for i in range(n)
        ]

    in_maps = []
    for c in range(NCORES):
        b, half = divmod(c, 2)
        lo, hi = half * IH, half * IH + IH
        perm = np.r_[lo:hi, 0:lo, hi:S]
        vresf = v_in[b, lo:hi].reshape(2, 4, 128, D).transpose(0, 2, 1, 3)
        m = {
            "w1T": w1t,
            "w2T": w2t,
            "w3T": w3t,
            "gamma": gam,
            "beta": bet,
        }
        for j, a in enumerate(chunks(_to_tiles_T(seq_q[b][perm], np.float16), 4)):
            m[f"sq{j}T"] = a
        for i, a in enumerate(chunks(_to_tiles_T(seq_k[b, lo:hi], np.float16), 2)):
            m[f"sk{i}T"] = a
        for j, a in enumerate(chunks(_to_tiles_T(v_in[b][perm], bf16), 4)):
            m[f"vin{j}T"] = a
        for j in range(2):
            m[f"vres{j}"] = np.ascontiguousarray(vresf[j]).astype(bf16)
        in_maps.append(m)

    res = bass_utils.run_bass_kernel_spmd(
        nc, in_maps, core_ids=list(range(NCORES)), trace=_trace
    )
    global _last_run
    _last_run = res

    full = np.empty((B, S, D), dtype=np.float32)
    for c in range(NCORES):
        b, half = divmod(c, 2)
        for it in range(ITILES):
            o = res.results[c][f"out{it}"]  # [128, 512]
            full[b, half * IH + 128 * it : half * IH + 128 * (it + 1)] = o
    return full


_last_run = None
